# revision 6
# baseline (speedup 1.0000x reference)
"""PointPillarScatter (intersweep, 3 bins) Trainium2 Bass kernel.

Problem: for each of 3 bins, scatter 64000 pillar rows [64 feats] into a
[B=4, C=64, NY=496, NX=432] fp32 canvas at (b, :, y, x); empty cells zero.

Strategy (8 NeuronCores, SPMD), v3 — fp16 transport, DMA'd fp8 masks:
  - Output travels as fp16 (41.1 MB/core; tolerance is 2e-2, fp16
    quantization is ~2.4e-4); features as a single fp16 term.
  - Each core owns 6 quarter-canvases = 3 pairs x 2 halves; a pair is 108
    windows of 496 cells (one PSUM bank). Per window one fp16 matmul:
      psum[128, 496] = lhsT[128, 128].T @ onehot[128, 496]
    K rows = 2 halves x 64 slots (max occupancy 41).
  - One-hot masks come from two sources to balance engines:
      * windows w%3 != 2: DVE tensor_scalar is_equal (4x perf mode, ~345ns)
      * windows w%3 == 2: precomputed on host, DMA'd as fp8 (exact 0/1;
        mixed fp16 lhsT x fp8 rhs matmul verified bit-exact on HW). This
        converts spare DMA bandwidth into DVE headroom.
  - lhsT is stored DENSE block-diagonal in DRAM ([128, W, 128] fp16, zeros
    included) so each chunk is ONE contiguous DMA, prefetched one chunk
    ahead (v2 loaded strided at chunk top -> ~4us PE stall per chunk).
  - No GPSIMD in steady state (its shared SBUF port would block DVE 4x
    masks -- the v1 failure mode).
  - PSUM tiles [128, 4, 512] f32 (4 banks; window j in bank j). Evac
    PSUM->SBUF casts to fp16 4 windows per op (FD=1984), split DVE/ACT.
  - 9 out-DMAs of 4.57 MB (sync ring) overlap in-DMAs (scalar ring).
"""

import numpy as np
import ml_dtypes

import concourse.bass as bass
import concourse.tile as tile
from concourse import bacc, mybir
from concourse.bass_utils import run_bass_kernel_spmd

# Problem geometry (hardcoded; kernel.py must be self-contained).
B = 4
C = 64
NX = 432
NY = 496
NBINS = 3
NCORES = 8

NQ = NBINS * B * 4          # 48 quarter-canvases
YQ = NY // 4                # 124 y-rows per quarter
QCELLS = YQ * NX            # 53568 cells per quarter
QPC = NQ // NCORES          # 6 quarters per core
PAIRS = QPC // 2            # 3 pairs per core
NW = 496                    # cells per window (<=512: one PSUM bank)
WPP = QCELLS // NW          # 108 windows per pair
WINDOWS = PAIRS * WPP       # 324 windows per core
CH = 36                     # windows per staging chunk / out-DMA
NCHUNKS = WPP // CH         # 3 chunks per pair
RP = 64                     # pillar slots per window per half (max seen 41)
PSW = 4                     # windows per PSUM tile (4 banks)
NPT = CH // PSW             # 9 psum tiles per chunk
MSKMOD = 3                  # windows w % MSKMOD == MSKMOD-1 use DMA'd fp8 masks
M8 = CH // MSKMOD           # DMA'd masks per chunk (12)

_cache = {}


def _dve_evac(chunk_idx, t):
    """Which psum-tile evacuations run on the DVE (rest on ACT)."""
    return t in (2, 5) or (t == 8 and chunk_idx % 2 == 0)


def _build():
    nc = bacc.Bacc(trn_type="TRN2")
    fp16 = mybir.dt.float16
    fp8 = mybir.dt.float8e4
    f32 = mybir.dt.float32
    lhst_d = nc.dram_tensor("lhst", [2 * RP, WINDOWS, 128], fp16,
                            kind="ExternalInput")
    iota_d = nc.dram_tensor("iotat", [2 * RP, NW], fp16, kind="ExternalInput")
    relc_d = nc.dram_tensor("relc", [2 * RP, WINDOWS], f32,
                            kind="ExternalInput")
    mask8_d = nc.dram_tensor("mask8", [PAIRS, NCHUNKS, 2 * RP, M8, NW], fp8,
                             kind="ExternalInput")
    out_d = nc.dram_tensor("out", [PAIRS, NCHUNKS, 128, CH, NW], fp16,
                           kind="ExternalOutput")

    chunks = [(pair, ch) for pair in range(PAIRS) for ch in range(NCHUNKS)]

    with tile.TileContext(nc) as tc:
        with (
            tc.tile_pool(name="const", bufs=1) as constp,
            tc.tile_pool(name="ltp", bufs=2) as ltp,
            tc.tile_pool(name="m8p", bufs=2) as m8p,
            tc.tile_pool(name="maskp", bufs=8) as maskp,
            tc.tile_pool(name="stage", bufs=2) as stagep,
            tc.tile_pool(name="psum", bufs=2, space=bass.MemorySpace.PSUM) as psump,
        ):
            iota = constp.tile([128, NW], fp16, name="iota")
            relc = constp.tile([128, WINDOWS], f32, name="relc")
            nc.scalar.dma_start(out=iota[:], in_=iota_d[:])
            nc.scalar.dma_start(out=relc[:], in_=relc_d[:])

            def load_chunk(ci):
                pair, ch = chunks[ci]
                g0 = pair * WPP + ch * CH
                lt = ltp.tile([128, CH, 128], fp16, name="lt")
                nc.scalar.dma_start(out=lt[:], in_=lhst_d[:, g0:g0 + CH, :])
                m8 = m8p.tile([128, M8, NW], fp8, name="m8")
                nc.scalar.dma_start(out=m8[:], in_=mask8_d[pair, ch])
                return lt, m8

            nxt = load_chunk(0)
            for ci, (pair, ch) in enumerate(chunks):
                g0 = pair * WPP + ch * CH
                lt, m8 = nxt
                if ci + 1 < len(chunks):
                    nxt = load_chunk(ci + 1)
                st = stagep.tile([128, CH, NW], fp16, name="st")
                for t in range(NPT):
                    pt = psump.tile([128, PSW, 512], f32, name="pt")
                    for j in range(PSW):
                        w = PSW * t + j
                        if w % MSKMOD == MSKMOD - 1:
                            rhs = m8[:, w // MSKMOD, :]
                        else:
                            mask = maskp.tile([128, NW], fp16, name="mask")
                            nc.vector.tensor_scalar(
                                out=mask[:],
                                in0=iota[:],
                                scalar1=relc[:, g0 + w:g0 + w + 1],
                                scalar2=None,
                                op0=mybir.AluOpType.is_equal,
                            )
                            rhs = mask[:]
                        nc.tensor.matmul(pt[:, j, 0:NW], lt[:, w, :], rhs,
                                         start=True, stop=True)
                    dst = st[:, PSW * t:PSW * (t + 1), :]
                    src = pt[:, :, 0:NW]
                    if _dve_evac(ci, t):
                        nc.vector.tensor_copy(out=dst, in_=src)
                    else:
                        nc.scalar.copy(out=dst, in_=src)
                nc.sync.dma_start(out=out_d[pair, ch], in_=st[:])
    nc.compile()
    return nc


def _pack(inputs):
    lhst = np.zeros((NCORES, 2 * RP, WINDOWS, 128), np.float16)
    iota = np.broadcast_to(np.arange(NW, dtype=np.float32),
                           (NCORES, 2 * RP, NW)).astype(np.float16)
    relc_a = np.full((NCORES, 2 * RP, WINDOWS), -1.0, np.float32)
    mask8 = np.zeros((NCORES, PAIRS, NCHUNKS, 2 * RP, M8, NW),
                     ml_dtypes.float8_e4m3fn)

    for bin_i in range(NBINS):
        feats = np.asarray(inputs[f"pillar_features_bin_{bin_i}"],
                           np.float32).astype(np.float16)
        coords = np.asarray(inputs[f"voxel_coords_bin_{bin_i}"])
        cb = np.asarray(coords[:, 0], np.int64)
        cy = np.asarray(coords[:, 2], np.int64)
        cx = np.asarray(coords[:, 3], np.int64)
        for b in range(B):
            rows_b = np.nonzero(cb == b)[0]
            y_b, x_b = cy[rows_b], cx[rows_b]
            for yq in range(4):
                q = bin_i * 16 + b * 4 + yq
                core, j = divmod(q, QPC)
                pair, half = divmod(j, 2)
                sel = (y_b >= YQ * yq) & (y_b < YQ * (yq + 1))
                rows = rows_b[sel]
                qcell = (y_b[sel] - YQ * yq) * NX + x_b[sel]
                w = qcell // NW
                rel = qcell % NW
                order = np.argsort(w, kind="stable")
                rows, w, rel = rows[order], w[order], rel[order]
                cnt = np.bincount(w, minlength=WPP)
                if cnt.max() > RP:
                    raise OverflowError(int(cnt.max()))
                off = np.concatenate([[0], np.cumsum(cnt)[:-1]])
                slot = np.arange(len(rows)) - off[w]
                r = half * RP + slot
                wins = pair * WPP + w
                lhst[core, r, wins, half * C:(half + 1) * C] = feats[rows]
                relc_a[core, r, wins] = rel
                # fp8 masks for DMA'd windows (w % MSKMOD == MSKMOD-1)
                dsel = (w % MSKMOD) == MSKMOD - 1
                if dsel.any():
                    wd = w[dsel]
                    mask8[core, pair, wd // CH, r[dsel], (wd % CH) // MSKMOD,
                          rel[dsel]] = 1.0
    return [{"lhst": lhst[c], "iotat": iota[c], "relc": relc_a[c],
             "mask8": mask8[c]} for c in range(NCORES)]


def _run(inputs, trace=False):
    if "nc" not in _cache:
        _cache["nc"] = _build()
    nc = _cache["nc"]
    in_maps = _pack(inputs)
    res = run_bass_kernel_spmd(nc, in_maps, core_ids=list(range(NCORES)),
                               trace=trace)
    outs = [np.zeros((B, C, NY, NX), np.float32) for _ in range(NBINS)]
    for core in range(NCORES):
        blk = np.asarray(res.results[core]["out"])  # [PAIRS,NCHUNKS,128,CH,NW]
        for pair in range(PAIRS):
            # [NCHUNKS, 128, CH, NW] -> [128, QCELLS]
            a = blk[pair].transpose(1, 0, 2, 3).reshape(128, QCELLS)
            for half in range(2):
                q = core * QPC + pair * 2 + half
                bin_i, rem = divmod(q, 16)
                b, yq = divmod(rem, 4)
                outs[bin_i][b, :, YQ * yq:YQ * (yq + 1), :] = (
                    a[half * C:(half + 1) * C]
                    .reshape(C, YQ, NX).astype(np.float32))
    return tuple(outs), res


def kernel(**inputs):
    out, _ = _run(inputs)
    return out


def kernel_traced(**inputs):
    """Like kernel() but also returns BassKernelResults (for test.py)."""
    return _run(inputs, trace=True)
